# revision 1
# baseline (speedup 1.0000x reference)
"""Chamfer distance (pytorch3d defaults) on 8 Trainium2 NeuronCores.

Problem: gts_X, pred_X: [4, 8192, 3] fp32. loss = mean_b mean_n min_p d(x_bn, y_bp)
                                              + mean_b mean_p min_n d(x_bn, y_bp),
d = squared euclidean distance. gts_normals is unused (reference default path).

Sharding: 8 independent tasks = 4 batches x 2 directions, one per core.
Each core computes per-query min_r d(Q_q, R_r) for its (Q, R) pair of
8192-point clouds; the host sums, guards, and averages.

Device algorithm per core:
- Both clouds are sorted by the z coordinate on the host. Each query
  super-block (4 row blocks of 128 sorted queries) only scans a WINDOW of
  WIN_TILES ref col-tiles (WIN_TILES*512 sorted refs) centered on its rank
  range.
  A query's true nearest neighbor can only be outside the window if the
  squared z-gap to the window edge is smaller than the found min; the host
  verifies that condition per query and recomputes the (rare/none) escapes
  exactly in numpy, so the result is exact for any input.
- d[q, r] = |Q|^2 + |R|^2 - 2 Q.R via ONE K=16 bf16 matmul per (128q x 512r)
  tile using an exact hi/lo bf16 split (bf16 products are exact in fp32, PSUM
  accumulates fp32 => ~fp32 precision).
- Matmuls are packed 4x with tile_position row groups.
- Min-reduction: DIRECT_POS col-tiles are min-reduced straight from PSUM by
  the DVE (1x mode); the rest are ACT-copied PSUM->SBUF with a bf16 downcast
  and folded by a DVE tensor_tensor min tree in 2x bf16 mode.
"""

import sys

sys.path.insert(0, "/opt/trn_rl_repo")

import numpy as np
import ml_dtypes

import concourse.bacc as bacc
import concourse.mybir as mybir
from concourse.tile import TileContext
from concourse.bass_utils import run_bass_kernel_spmd

BF16 = ml_dtypes.bfloat16

B = 4
N = 8192
K = 16  # contraction rows after hi/lo split
MBLK = 128  # queries per row block (PSUM partitions)
NBLK = 512  # refs per matmul (one PSUM bank of fp32)
NMB = N // MBLK  # 64 row blocks
NNB = N // NBLK  # 16 col tiles
SB = NMB // 4  # 16 super-blocks of 4 row blocks

WIN_TILES = 3  # ref col-tiles scanned per super-block
# within-window positions reduced directly from PSUM by the DVE (interleaved
# with ACT-copied positions so the PSUM-slot release chain alternates engines)
DIRECT_POS = (1,)
ACT_POS = tuple(t for t in range(WIN_TILES) if t not in DIRECT_POS)
DIRECT_COLS = len(DIRECT_POS)
ACT_COLS = len(ACT_POS)
BF1 = ACT_COLS  # all ACT cols fold through ONE tree (fewer DVE ops)

LAST_RESULTS = None  # BassKernelResults of the most recent run (for test.py)


def _win_start(s):
    """First ref col-tile of super-block s's window (rank-based, static)."""
    return min(max(s - WIN_TILES // 2, 0), NNB - WIN_TILES)


def _tt_min(nc, out, a, b):
    nc.vector.tensor_tensor(out, a, b, op=mybir.AluOpType.min)


def _half_tree(nc, work_pool, bfb, ncols, part_col):
    """Fold bfb [128, 4, ncols*512] bf16 down to part_col [128, 4, 1] fp32
    via 2x-mode TT mins + one short 1x reduce. szX are per-block element
    counts."""
    sz1 = ncols * 512 // 2  # per-block run after level 1
    sz2 = sz1 // 2
    sz3 = sz2 // 2
    sz4 = sz3 // 2
    t1 = work_pool.tile([MBLK, 4, sz1], mybir.dt.bfloat16, tag="t1")
    t2 = work_pool.tile([MBLK, 4, sz2], mybir.dt.bfloat16, tag="t2")
    t3 = work_pool.tile([MBLK, 4, sz3], mybir.dt.bfloat16, tag="t3")
    t4 = work_pool.tile([MBLK, 4, sz4], mybir.dt.bfloat16, tag="t4")
    _tt_min(nc, t1[:], bfb[:, :, 0:sz1], bfb[:, :, sz1 : 2 * sz1])
    _tt_min(nc, t2[:], t1[:, :, 0:sz2], t1[:, :, sz2 : 2 * sz2])
    _tt_min(nc, t3[:], t2[:, :, 0:sz3], t2[:, :, sz3 : 2 * sz3])
    _tt_min(nc, t4[:], t3[:, :, 0:sz4], t3[:, :, sz4 : 2 * sz4])
    nc.vector.tensor_reduce(
        part_col, t4[:], axis=mybir.AxisListType.X, op=mybir.AluOpType.min
    )


def _build_bass():
    nc = bacc.Bacc("TRN2")
    lhs = nc.dram_tensor("lhs", [K, N], mybir.dt.bfloat16, kind="ExternalInput")
    rhs = nc.dram_tensor("rhs", [K, N], mybir.dt.bfloat16, kind="ExternalInput")
    out = nc.dram_tensor("out", [MBLK, NMB], mybir.dt.float32, kind="ExternalOutput")

    with TileContext(nc) as tc:
        with (
            tc.tile_pool(name="data", bufs=1) as data_pool,
            tc.tile_pool(name="work", bufs=4) as work_pool,
            tc.tile_pool(name="ps", bufs=4, space="PSUM") as ps_pool,
        ):
            # operands replicated at partition offsets 0/32/64/96 so four
            # row-group-packed matmuls can run concurrently
            lhs_sb = data_pool.tile([128, N], mybir.dt.bfloat16)
            rhs_sb = data_pool.tile([128, N], mybir.dt.bfloat16)
            for g in range(4):
                nc.sync.dma_start(lhs_sb[32 * g : 32 * g + K, :], lhs.ap())
                nc.sync.dma_start(rhs_sb[32 * g : 32 * g + K, :], rhs.ap())

            blockmins = data_pool.tile([MBLK, NMB], mybir.dt.float32)

            for s in range(SB):
                w0 = _win_start(s)
                part = work_pool.tile(
                    [MBLK, 4, DIRECT_COLS + 1], mybir.dt.float32, tag="part"
                )
                bfb1 = work_pool.tile(
                    [MBLK, 4, BF1 * 512], mybir.dt.bfloat16, tag="bfb1"
                )
                for t in range(WIN_TILES):
                    n = w0 + t
                    # two 2-bank PSUM tiles per col (blocks 0-1 and 2-3) so
                    # the pool has 4 slots in flight and consumers split into
                    # shorter units -> less head-of-line blocking
                    ps_a = ps_pool.tile([MBLK, 2, NBLK], mybir.dt.float32, tag="ps")
                    ps_b = ps_pool.tile([MBLK, 2, NBLK], mybir.dt.float32, tag="ps")
                    pshalves = [ps_a, ps_b]
                    for j in range(4):
                        m = 4 * s + j
                        nc.tensor.matmul(
                            pshalves[j // 2][:, j % 2, :],
                            lhs_sb[32 * j : 32 * j + K, m * MBLK : (m + 1) * MBLK],
                            rhs_sb[32 * j : 32 * j + K, n * NBLK : (n + 1) * NBLK],
                            start=True,
                            stop=True,
                            tile_position=(32 * j, 0),
                        )
                    if t in DIRECT_POS:
                        for h in range(2):
                            nc.vector.tensor_reduce(
                                part[:, 2 * h : 2 * h + 2, DIRECT_POS.index(t)],
                                pshalves[h][:],
                                axis=mybir.AxisListType.X,
                                op=mybir.AluOpType.min,
                            )
                    else:
                        co = ACT_POS.index(t) * 512
                        for h in range(2):
                            nc.scalar.copy(
                                bfb1[:, 2 * h : 2 * h + 2, co : co + 512],
                                pshalves[h][:],
                            )
                    if t == ACT_POS[-1]:
                        _half_tree(
                            nc, work_pool, bfb1, BF1, part[:, :, DIRECT_COLS]
                        )
                nc.vector.tensor_reduce(
                    blockmins[:, 4 * s : 4 * s + 4],
                    part[:],
                    axis=mybir.AxisListType.X,
                    op=mybir.AluOpType.min,
                )

            nc.sync.dma_start(out.ap(), blockmins[:])
    return nc


def _split_bf16(v):
    """v (fp32) ~= hi + lo with both bf16; residual is O(2^-18 |v|)."""
    hi = v.astype(BF16)
    lo = (v - hi.astype(np.float32)).astype(BF16)
    return hi, lo


def _prep_core_inputs(Q, R):
    """Build the K=16 lhsT (queries) and rhs (refs) bf16 matrices so that
    lhsT.T @ rhs accumulated in fp32 equals |Q|^2 + |R|^2 - 2 Q.R."""
    Qh, Ql = _split_bf16(Q)  # [N, 3]
    Rh, Rl = _split_bf16(-2.0 * R)  # [N, 3]
    nQh, nQl = _split_bf16((Q * Q).sum(axis=1))  # [N]
    nRh, nRl = _split_bf16((R * R).sum(axis=1))  # [N]
    one = np.ones(N, dtype=BF16)

    L = np.empty([K, N], dtype=BF16)
    L[0:3] = Qh.T
    L[3:6] = Qh.T
    L[6:9] = Ql.T
    L[9:12] = Ql.T
    L[12] = nQh
    L[13] = nQl
    L[14] = one
    L[15] = one

    Rm = np.empty([K, N], dtype=BF16)
    Rm[0:3] = Rh.T
    Rm[3:6] = Rl.T
    Rm[6:9] = Rh.T
    Rm[9:12] = Rl.T
    Rm[12] = one
    Rm[13] = one
    Rm[14] = nRh
    Rm[15] = nRl
    return L, Rm


def _try_axon_reset():
    """The axon-tunneled device sporadically wedges (NRT_EXEC_UNIT_UNRECOVERABLE);
    axon_reset() recovers it."""
    try:
        import ctypes

        import jax

        jax.devices()
        lib = ctypes.CDLL("/opt/axon/libaxon_pjrt.so")
        lib.axon_reset.restype = ctypes.c_int64
        lib.axon_reset()
    except Exception:
        pass


def _task_pairs(gts_X, pred_X):
    for b in range(B):
        yield gts_X[b], pred_X[b]  # each gts point -> nearest pred
        yield pred_X[b], gts_X[b]  # each pred point -> nearest gts


def kernel(gts_X, pred_X, gts_normals=None, **_ignored):
    global LAST_RESULTS
    gts_X = np.asarray(gts_X, dtype=np.float32)
    pred_X = np.asarray(pred_X, dtype=np.float32)
    assert gts_X.shape == (B, N, 3) and pred_X.shape == (B, N, 3)

    in_maps = []
    sorted_pairs = []
    for Qr, Rr in _task_pairs(gts_X, pred_X):
        Qs = np.ascontiguousarray(Qr[np.argsort(Qr[:, 2], kind="stable")])
        Rs = np.ascontiguousarray(Rr[np.argsort(Rr[:, 2], kind="stable")])
        sorted_pairs.append((Qs, Rs))
        L, Rm = _prep_core_inputs(Qs, Rs)
        in_maps.append({"lhs": L, "rhs": Rm})

    nc = _build_bass()
    nc.finalize()
    res = None
    for attempt in range(3):
        try:
            res = run_bass_kernel_spmd(nc, in_maps, core_ids=list(range(8)))
            break
        except Exception:
            if attempt == 2:
                raise
            _try_axon_reset()
    LAST_RESULTS = res

    total = 0.0
    for (Qs, Rs), r in zip(sorted_pairs, res.results):
        mins = r["out"].astype(np.float64)  # [128, 64]; query rank = m*128 + p
        mins = mins.T.reshape(-1)  # rank-ordered per-query windowed mins
        # exactness guard: the true NN can only lie outside the window if the
        # squared z-gap to the window edge is below the windowed min
        s_idx = np.arange(N) // (4 * MBLK)
        w0 = np.array([_win_start(int(s)) for s in range(SB)])[s_idx]
        lo = w0 * NBLK  # first ref rank in window
        hi = lo + WIN_TILES * NBLK  # one past last
        zq = Qs[:, 2].astype(np.float64)
        zr = Rs[:, 2].astype(np.float64)
        gap_lo = np.where(lo > 0, zq - zr[np.maximum(lo - 1, 0)], np.inf)
        gap_hi = np.where(hi < N, zr[np.minimum(hi, N - 1)] - zq, np.inf)
        guard = np.minimum(gap_lo, gap_hi) ** 2
        bad = np.nonzero(mins > guard)[0]
        if len(bad):
            Qb = Qs[bad].astype(np.float64)
            d = ((Qb[:, None, :] - Rs[None, :, :].astype(np.float64)) ** 2).sum(-1)
            mins[bad] = d.min(axis=1)
        total += mins.sum()

    loss = total / (B * N)
    return np.asarray(loss, dtype=np.float32)



# revision 3
# speedup vs baseline: 1.1279x; 1.1279x over previous
"""Chamfer distance (pytorch3d defaults) on 8 Trainium2 NeuronCores.

Problem: gts_X, pred_X: [4, 8192, 3] fp32. loss = mean_b mean_n min_p d(x_bn, y_bp)
                                              + mean_b mean_p min_n d(x_bn, y_bp),
d = squared euclidean distance. gts_normals is unused (reference default path).

Sharding: 8 independent tasks = 4 batches x 2 directions, one per core.
Each core computes per-query windowed min over a 1024-wide, per-row-block
centered window of z-sorted refs; the host certifies each query with a z-gap
guard and recomputes the (~10%) uncertified queries exactly in numpy, so the
result is exact for any input.

Device algorithm per core (v2b):
- d[q, r] = |Q|^2 + |R|^2 - 2 Q.R via ONE K=16 bf16 matmul per (128q x 512r)
  tile using an exact hi/lo bf16 split (~fp32 precision in PSUM).
- Per 128-query row block m: window = refs [lo_m, lo_m+1024) centered on the
  block's rank range -> 2 matmuls into a [128, 2, 512] PSUM tile.
- PSUM drain (the wall: only DVE and ACT can read PSUM, ~1 elem/cycle/lane):
  blocks are split between engines to balance:
  - A-blocks: ACT copies the PSUM pair to bf16 SBUF; DVE then folds all of a
    super-block's A-copies with scalar_tensor_tensor min (4x mode on packed
    bf16) + one small tensor_reduce -> ~0.29 DVE ns/elem.
  - D-blocks: one DVE tensor_reduce XY straight off PSUM -> blockmins[:, m].
  Pattern: 9 super-blocks run [A,A,A,D], 7 run [A,A,D,D] (41 A / 23 D),
  balancing ACT ~39us vs DVE ~40us.
"""

import sys

sys.path.insert(0, "/opt/trn_rl_repo")

import numpy as np
import ml_dtypes

import concourse.bacc as bacc
import concourse.mybir as mybir
from concourse.tile import TileContext
from concourse.bass_utils import run_bass_kernel_spmd

BF16 = ml_dtypes.bfloat16

B = 4
N = 8192
K = 16  # contraction rows after hi/lo split
MBLK = 128  # queries per row block (PSUM partitions)
NBLK = 512  # refs per matmul (one PSUM bank of fp32)
NMB = N // MBLK  # 64 row blocks
SB = NMB // 4  # 16 super-blocks of 4 row blocks
W = 1024  # refs scanned per query block (2 PSUM banks)

# per-row-block window start (centered on the block's rank range)
LOS = [min(max(128 * m + 64 - W // 2, 0), N - W) for m in range(NMB)]

# number of ACT-consumed (A) blocks per super-block; the rest are DVE-direct.
# A-blocks are the leading blocks of each super-block.
NA_PER_SB = [3 if s % 16 in (0, 2, 4, 6, 8, 10, 12, 14, 15) else 2 for s in range(SB)]

MIN_BIG = 1e30

LAST_RESULTS = None  # BassKernelResults of the most recent run (for test.py)


def _build_bass():
    nc = bacc.Bacc("TRN2")
    lhs = nc.dram_tensor("lhs", [K, N], mybir.dt.bfloat16, kind="ExternalInput")
    rhs = nc.dram_tensor("rhs", [K, N], mybir.dt.bfloat16, kind="ExternalInput")
    out = nc.dram_tensor("out", [MBLK, NMB], mybir.dt.float32, kind="ExternalOutput")
    mn = mybir.AluOpType.min

    with TileContext(nc) as tc:
        with (
            tc.tile_pool(name="data", bufs=1) as data_pool,
            tc.tile_pool(name="work", bufs=2) as work_pool,
            tc.tile_pool(name="ps", bufs=4, space="PSUM") as ps_pool,
        ):
            lhs_sb = data_pool.tile([K, N], mybir.dt.bfloat16)
            rhs_sb = data_pool.tile([K, N], mybir.dt.bfloat16)
            nc.sync.dma_start(lhs_sb[:], lhs.ap())
            nc.sync.dma_start(rhs_sb[:], rhs.ap())

            blockmins = data_pool.tile([MBLK, NMB], mybir.dt.float32)

            for s in range(SB):
                na = NA_PER_SB[s]
                bf = work_pool.tile(
                    [MBLK, na, 2, NBLK], mybir.dt.bfloat16, tag="bf"
                )
                for j in range(4):
                    m = 4 * s + j
                    lo = LOS[m]
                    ps = ps_pool.tile([MBLK, 2, NBLK], mybir.dt.float32, tag="ps")
                    for h in range(2):
                        nc.tensor.matmul(
                            ps[:, h, :],
                            lhs_sb[:, m * MBLK : (m + 1) * MBLK],
                            rhs_sb[:, lo + h * NBLK : lo + (h + 1) * NBLK],
                            start=True,
                            stop=True,
                        )
                    if j < na:  # A-block: ACT drains PSUM to bf16
                        nc.scalar.copy(bf[:, j], ps[:])
                    else:  # D-block: DVE drains PSUM directly
                        nc.vector.tensor_reduce(
                            blockmins[:, m : m + 1],
                            ps[:],
                            axis=mybir.AxisListType.XY,
                            op=mn,
                        )
                # fold the na A-copies: 4 stt min levels (4x bf16) + one TR
                t1 = work_pool.tile([MBLK, na, 2, 256], mybir.dt.bfloat16, tag="t1")
                t2 = work_pool.tile([MBLK, na, 2, 128], mybir.dt.bfloat16, tag="t2")
                t3 = work_pool.tile([MBLK, na, 2, 64], mybir.dt.bfloat16, tag="t3")
                t4 = work_pool.tile([MBLK, na, 2, 32], mybir.dt.bfloat16, tag="t4")
                nc.vector.scalar_tensor_tensor(
                    t1[:], bf[:, :, :, 0:256], MIN_BIG, bf[:, :, :, 256:512],
                    op0=mn, op1=mn,
                )
                nc.vector.scalar_tensor_tensor(
                    t2[:], t1[:, :, :, 0:128], MIN_BIG, t1[:, :, :, 128:256],
                    op0=mn, op1=mn,
                )
                nc.vector.scalar_tensor_tensor(
                    t3[:], t2[:, :, :, 0:64], MIN_BIG, t2[:, :, :, 64:128],
                    op0=mn, op1=mn,
                )
                nc.vector.scalar_tensor_tensor(
                    t4[:], t3[:, :, :, 0:32], MIN_BIG, t3[:, :, :, 32:64],
                    op0=mn, op1=mn,
                )
                nc.vector.tensor_reduce(
                    blockmins[:, 4 * s : 4 * s + na],
                    t4[:],
                    axis=mybir.AxisListType.XY,
                    op=mn,
                )

            nc.sync.dma_start(out.ap(), blockmins[:])
    return nc


def _split_bf16(v):
    """v (fp32) ~= hi + lo with both bf16; residual is O(2^-18 |v|)."""
    hi = v.astype(BF16)
    lo = (v - hi.astype(np.float32)).astype(BF16)
    return hi, lo


def _prep_core_inputs(Q, R):
    """Build the K=16 lhsT (queries) and rhs (refs) bf16 matrices so that
    lhsT.T @ rhs accumulated in fp32 equals |Q|^2 + |R|^2 - 2 Q.R."""
    Qh, Ql = _split_bf16(Q)  # [N, 3]
    Rh, Rl = _split_bf16(-2.0 * R)  # [N, 3]
    nQh, nQl = _split_bf16((Q * Q).sum(axis=1))  # [N]
    nRh, nRl = _split_bf16((R * R).sum(axis=1))  # [N]
    one = np.ones(N, dtype=BF16)

    L = np.empty([K, N], dtype=BF16)
    L[0:3] = Qh.T
    L[3:6] = Qh.T
    L[6:9] = Ql.T
    L[9:12] = Ql.T
    L[12] = nQh
    L[13] = nQl
    L[14] = one
    L[15] = one

    Rm = np.empty([K, N], dtype=BF16)
    Rm[0:3] = Rh.T
    Rm[3:6] = Rl.T
    Rm[6:9] = Rh.T
    Rm[9:12] = Rl.T
    Rm[12] = one
    Rm[13] = one
    Rm[14] = nRh
    Rm[15] = nRl
    return L, Rm


def _try_axon_reset():
    """The axon-tunneled device sporadically wedges (NRT_EXEC_UNIT_UNRECOVERABLE);
    axon_reset() recovers it."""
    try:
        import ctypes

        import jax

        jax.devices()
        lib = ctypes.CDLL("/opt/axon/libaxon_pjrt.so")
        lib.axon_reset.restype = ctypes.c_int64
        lib.axon_reset()
    except Exception:
        pass


def _task_pairs(gts_X, pred_X):
    for b in range(B):
        yield gts_X[b], pred_X[b]  # each gts point -> nearest pred
        yield pred_X[b], gts_X[b]  # each pred point -> nearest gts


def kernel(gts_X, pred_X, gts_normals=None, **_ignored):
    global LAST_RESULTS
    gts_X = np.asarray(gts_X, dtype=np.float32)
    pred_X = np.asarray(pred_X, dtype=np.float32)
    assert gts_X.shape == (B, N, 3) and pred_X.shape == (B, N, 3)

    in_maps = []
    sorted_pairs = []
    for Qr, Rr in _task_pairs(gts_X, pred_X):
        Qs = np.ascontiguousarray(Qr[np.argsort(Qr[:, 2], kind="stable")])
        Rs = np.ascontiguousarray(Rr[np.argsort(Rr[:, 2], kind="stable")])
        sorted_pairs.append((Qs, Rs))
        L, Rm = _prep_core_inputs(Qs, Rs)
        in_maps.append({"lhs": L, "rhs": Rm})

    nc = _build_bass()
    nc.finalize()
    res = None
    for attempt in range(3):
        try:
            res = run_bass_kernel_spmd(nc, in_maps, core_ids=list(range(8)))
            break
        except Exception:
            if attempt == 2:
                raise
            _try_axon_reset()
    LAST_RESULTS = res

    los = np.array(LOS)
    q_idx = np.arange(N)
    lo = los[q_idx // MBLK]  # per-query window start
    hi = lo + W

    total = 0.0
    for (Qs, Rs), r in zip(sorted_pairs, res.results):
        mins = r["out"].astype(np.float64)  # [128, 64]; query rank = m*128 + p
        mins = mins.T.reshape(-1)  # rank-ordered per-query windowed mins
        # certification: the true NN can only lie outside the window if the
        # squared z-gap to the window edge is below the windowed min; pad the
        # compare for the bf16 downcast of the A-block path (rel 2^-8).
        zq = Qs[:, 2].astype(np.float64)
        zr = Rs[:, 2].astype(np.float64)
        gap_lo = np.where(lo > 0, zq - zr[np.maximum(lo - 1, 0)], np.inf)
        gap_hi = np.where(hi < N, zr[np.minimum(hi, N - 1)] - zq, np.inf)
        guard = np.minimum(gap_lo, gap_hi) ** 2
        bad = np.nonzero(mins > guard * (1.0 - 2.0**-7))[0]
        if len(bad):
            Qb = Qs[bad].astype(np.float64)
            Rd = Rs.astype(np.float64)
            nq = (Qb * Qb).sum(1)
            nr = (Rd * Rd).sum(1)
            d = nq[:, None] + nr[None, :] - 2.0 * (Qb @ Rd.T)
            mins[bad] = d.min(axis=1)
        total += mins.sum()

    loss = total / (B * N)
    return np.asarray(loss, dtype=np.float32)


# revision 4
# speedup vs baseline: 1.5944x; 1.4137x over previous
"""Chamfer distance (pytorch3d defaults) on 8 Trainium2 NeuronCores.

Problem: gts_X, pred_X: [4, 8192, 3] fp32. loss = mean_b mean_n min_p d(x_bn, y_bp)
                                              + mean_b mean_p min_n d(x_bn, y_bp),
d = squared euclidean distance. gts_normals is unused (reference default path).

Sharding: 8 independent tasks = 4 batches x 2 directions, one per core.
Each core computes per-query windowed min over a 1024-wide, per-row-block
centered window of z-sorted refs; the host certifies each query with a z-gap
guard and recomputes the uncertified queries exactly in numpy.

Device algorithm per core (v2c):
- d[q, r] = |Q|^2 + |R|^2 - 2 Q.R via ONE K=16 bf16 matmul per (128q x 512r)
  tile using an exact hi/lo bf16 split (~fp32 precision in PSUM). Matmuls are
  packed 4x with tile_position row groups (keeps the PE at the 267ns/tile
  fused-weight-load pace; unpacked they cost 618+134ns).
- Per 128-query row block m: window = refs [lo_m, lo_m+1024) -> 2 matmuls
  into a [128, 2, 512] PSUM tile.
- PSUM drain (the wall: only DVE and ACT can read PSUM, ~1 elem/cycle/lane):
  - S-blocks (even m): ONE ACT op: out=exp(-BETA*d) with accum_out giving
    S_q = sum_r exp(-BETA * d_qr); the host recovers the windowed softmin
    -ln(S)/BETA (bias ~ -1e-5, validated under the 2e-2 tolerance; S==0 /
    tiny-S queries are recomputed exactly on host, as are guard escapes).
  - D-blocks (odd m): ONE DVE tensor_reduce XY straight off PSUM -> exact min.
  Each engine drains half the elements with zero cross-engine coupling.
"""

import sys

sys.path.insert(0, "/opt/trn_rl_repo")

import numpy as np
import ml_dtypes

import concourse.bacc as bacc
import concourse.mybir as mybir
from concourse.tile import TileContext
from concourse.bass_utils import run_bass_kernel_spmd

BF16 = ml_dtypes.bfloat16

B = 4
N = 8192
K = 16  # contraction rows after hi/lo split
MBLK = 128  # queries per row block (PSUM partitions)
NBLK = 512  # refs per matmul (one PSUM bank of fp32)
NMB = N // MBLK  # 64 row blocks
SB = NMB // 4  # 16 super-blocks of 4 row blocks
W = 1024  # refs scanned per query block (2 PSUM banks)

# per-row-block window start (centered on the block's rank range)
LOS = [min(max(128 * m + 64 - W // 2, 0), N - W) for m in range(NMB)]

BETA = 2500.0  # softmin sharpness (squared-distance units)
S_MIN = float(np.exp(-75.0))  # below this the softmin is underflow-suspect


def _is_soft(m):
    return m % 2 == 0


LAST_RESULTS = None  # BassKernelResults of the most recent run (for test.py)


def _build_bass():
    nc = bacc.Bacc("TRN2")
    lhs = nc.dram_tensor("lhs", [K, N], mybir.dt.bfloat16, kind="ExternalInput")
    rhs = nc.dram_tensor("rhs", [K, N], mybir.dt.bfloat16, kind="ExternalInput")
    out = nc.dram_tensor("out", [MBLK, NMB], mybir.dt.float32, kind="ExternalOutput")
    mn = mybir.AluOpType.min

    with TileContext(nc) as tc:
        with (
            tc.tile_pool(name="data", bufs=1) as data_pool,
            tc.tile_pool(name="work", bufs=4) as work_pool,
            tc.tile_pool(name="ps", bufs=4, space="PSUM") as ps_pool,
        ):
            # operands replicated at partition offsets 0/32/64/96 so four
            # row-group-packed matmuls can run concurrently
            lhs_sb = data_pool.tile([128, N], mybir.dt.bfloat16)
            rhs_sb = data_pool.tile([128, N], mybir.dt.bfloat16)
            for g in range(4):
                nc.sync.dma_start(lhs_sb[32 * g : 32 * g + K, :], lhs.ap())
                nc.sync.dma_start(rhs_sb[32 * g : 32 * g + K, :], rhs.ap())

            arena = data_pool.tile([MBLK, NMB], mybir.dt.float32)

            for s in range(SB):
                for j in range(4):
                    m = 4 * s + j
                    lo = LOS[m]
                    ps = ps_pool.tile([MBLK, 2, NBLK], mybir.dt.float32, tag="ps")
                    for h in range(2):
                        nc.tensor.matmul(
                            ps[:, h, :],
                            lhs_sb[32 * j : 32 * j + K, m * MBLK : (m + 1) * MBLK],
                            rhs_sb[32 * j : 32 * j + K, lo + h * NBLK : lo + (h + 1) * NBLK],
                            start=True,
                            stop=True,
                            tile_position=(32 * j, 0),
                        )
                    if _is_soft(m):  # S-block: ACT softmin (exp + sum-accum)
                        scratch = work_pool.tile(
                            [MBLK, 2, NBLK], mybir.dt.bfloat16, tag="sc"
                        )
                        nc.scalar.activation(
                            scratch[:],
                            ps[:],
                            mybir.ActivationFunctionType.Exp,
                            bias=0.0,
                            scale=-BETA,
                            accum_out=arena[:, m : m + 1],
                        )
                    else:  # D-block: DVE exact min straight off PSUM
                        nc.vector.tensor_reduce(
                            arena[:, m : m + 1],
                            ps[:],
                            axis=mybir.AxisListType.XY,
                            op=mn,
                        )

            nc.sync.dma_start(out.ap(), arena[:])
    return nc


def _split_bf16(v):
    """v (fp32) ~= hi + lo with both bf16; residual is O(2^-18 |v|)."""
    hi = v.astype(BF16)
    lo = (v - hi.astype(np.float32)).astype(BF16)
    return hi, lo


def _prep_core_inputs(Q, R):
    """Build the K=16 lhsT (queries) and rhs (refs) bf16 matrices so that
    lhsT.T @ rhs accumulated in fp32 equals |Q|^2 + |R|^2 - 2 Q.R."""
    Qh, Ql = _split_bf16(Q)  # [N, 3]
    Rh, Rl = _split_bf16(-2.0 * R)  # [N, 3]
    nQh, nQl = _split_bf16((Q * Q).sum(axis=1))  # [N]
    nRh, nRl = _split_bf16((R * R).sum(axis=1))  # [N]
    one = np.ones(N, dtype=BF16)

    L = np.empty([K, N], dtype=BF16)
    L[0:3] = Qh.T
    L[3:6] = Qh.T
    L[6:9] = Ql.T
    L[9:12] = Ql.T
    L[12] = nQh
    L[13] = nQl
    L[14] = one
    L[15] = one

    Rm = np.empty([K, N], dtype=BF16)
    Rm[0:3] = Rh.T
    Rm[3:6] = Rl.T
    Rm[6:9] = Rh.T
    Rm[9:12] = Rl.T
    Rm[12] = one
    Rm[13] = one
    Rm[14] = nRh
    Rm[15] = nRl
    return L, Rm


def _try_axon_reset():
    """The axon-tunneled device sporadically wedges (NRT_EXEC_UNIT_UNRECOVERABLE);
    axon_reset() recovers it."""
    try:
        import ctypes

        import jax

        jax.devices()
        lib = ctypes.CDLL("/opt/axon/libaxon_pjrt.so")
        lib.axon_reset.restype = ctypes.c_int64
        lib.axon_reset()
    except Exception:
        pass


def _task_pairs(gts_X, pred_X):
    for b in range(B):
        yield gts_X[b], pred_X[b]  # each gts point -> nearest pred
        yield pred_X[b], gts_X[b]  # each pred point -> nearest gts


def kernel(gts_X, pred_X, gts_normals=None, **_ignored):
    global LAST_RESULTS
    gts_X = np.asarray(gts_X, dtype=np.float32)
    pred_X = np.asarray(pred_X, dtype=np.float32)
    assert gts_X.shape == (B, N, 3) and pred_X.shape == (B, N, 3)

    in_maps = []
    sorted_pairs = []
    for Qr, Rr in _task_pairs(gts_X, pred_X):
        Qs = np.ascontiguousarray(Qr[np.argsort(Qr[:, 2], kind="stable")])
        Rs = np.ascontiguousarray(Rr[np.argsort(Rr[:, 2], kind="stable")])
        sorted_pairs.append((Qs, Rs))
        L, Rm = _prep_core_inputs(Qs, Rs)
        in_maps.append({"lhs": L, "rhs": Rm})

    nc = _build_bass()
    nc.finalize()
    res = None
    for attempt in range(3):
        try:
            res = run_bass_kernel_spmd(nc, in_maps, core_ids=list(range(8)))
            break
        except Exception:
            if attempt == 2:
                raise
            _try_axon_reset()
    LAST_RESULTS = res

    los = np.array(LOS)
    q_idx = np.arange(N)
    lo = los[q_idx // MBLK]  # per-query window start
    hi = lo + W
    soft = np.array([_is_soft(m) for m in range(NMB)])[q_idx // MBLK]

    total = 0.0
    for (Qs, Rs), r in zip(sorted_pairs, res.results):
        vals = r["out"].astype(np.float64)  # [128, 64]; query rank = m*128 + p
        vals = vals.T.reshape(-1)  # rank-ordered per-query S or min
        mins = np.where(
            soft,
            -np.log(np.maximum(vals, 1e-300)) / BETA,  # softmin recovery
            vals,
        )
        # certification: true NN outside the window only if the squared z-gap
        # to the window edge is below the windowed min (pad for softmin bias /
        # exp-table error); softmin underflow (tiny S) is also uncertified.
        zq = Qs[:, 2].astype(np.float64)
        zr = Rs[:, 2].astype(np.float64)
        gap_lo = np.where(lo > 0, zq - zr[np.maximum(lo - 1, 0)], np.inf)
        gap_hi = np.where(hi < N, zr[np.minimum(hi, N - 1)] - zq, np.inf)
        guard = np.minimum(gap_lo, gap_hi) ** 2
        bad = (mins > guard * (1.0 - 2.0**-7)) | (soft & (vals < S_MIN))
        bad = np.nonzero(bad)[0]
        if len(bad):
            Qb = Qs[bad].astype(np.float64)
            Rd = Rs.astype(np.float64)
            nq = (Qb * Qb).sum(1)
            nr = (Rd * Rd).sum(1)
            d = nq[:, None] + nr[None, :] - 2.0 * (Qb @ Rd.T)
            mins[bad] = d.min(axis=1)
        total += mins.sum()

    loss = total / (B * N)
    return np.asarray(loss, dtype=np.float32)


# revision 5
# speedup vs baseline: 1.6015x; 1.0044x over previous
"""Chamfer distance (pytorch3d defaults) on 8 Trainium2 NeuronCores.

Problem: gts_X, pred_X: [4, 8192, 3] fp32. loss = mean_b mean_n min_p d(x_bn, y_bp)
                                              + mean_b mean_p min_n d(x_bn, y_bp),
d = squared euclidean distance. gts_normals is unused (reference default path).

Sharding: 8 independent tasks = 4 batches x 2 directions, one per core.
Each core computes per-query windowed min over a 1024-wide, per-row-block
centered window of z-sorted refs; the host certifies each query with a z-gap
guard and recomputes the uncertified queries exactly in numpy.

Device algorithm per core (v2c):
- d[q, r] = |Q|^2 + |R|^2 - 2 Q.R via ONE K=16 bf16 matmul per (128q x 512r)
  tile using an exact hi/lo bf16 split (~fp32 precision in PSUM). Matmuls are
  packed 4x with tile_position row groups (keeps the PE at the 267ns/tile
  fused-weight-load pace; unpacked they cost 618+134ns).
- Per 128-query row block m: window = refs [lo_m, lo_m+1024) -> 2 matmuls
  into a [128, 2, 512] PSUM tile.
- PSUM drain (the wall: only DVE and ACT can read PSUM, ~1 elem/cycle/lane):
  - S-blocks (even m): ONE ACT op: out=exp(-BETA*d) with accum_out giving
    S_q = sum_r exp(-BETA * d_qr); the host recovers the windowed softmin
    -ln(S)/BETA (bias ~ -1e-5, validated under the 2e-2 tolerance; S==0 /
    tiny-S queries are recomputed exactly on host, as are guard escapes).
  - D-blocks (odd m): ONE DVE tensor_reduce XY straight off PSUM -> exact min.
  Each engine drains half the elements with zero cross-engine coupling.
"""

import sys

sys.path.insert(0, "/opt/trn_rl_repo")

import numpy as np
import ml_dtypes

import concourse.bacc as bacc
import concourse.mybir as mybir
from concourse.tile import TileContext
from concourse.bass_utils import run_bass_kernel_spmd

BF16 = ml_dtypes.bfloat16

B = 4
N = 8192
K = 16  # contraction rows after hi/lo split
MBLK = 128  # queries per row block (PSUM partitions)
NBLK = 512  # refs per matmul (one PSUM bank of fp32)
NMB = N // MBLK  # 64 row blocks
SB = NMB // 4  # 16 super-blocks of 4 row blocks
TAIL = 8  # blocks on each end that scan half-width windows
WS = [512 if (m < TAIL or m >= NMB - TAIL) else 1024 for m in range(NMB)]

# per-row-block window start (centered on the block's rank range)
LOS = [min(max(128 * m + 64 - WS[m] // 2, 0), N - WS[m]) for m in range(NMB)]

BETA = 2500.0  # softmin sharpness (squared-distance units)
S_MIN = float(np.exp(-75.0))  # below this the softmin is underflow-suspect


def _is_soft(m):
    return m % 2 == 0 and m not in (28, 36)


LAST_RESULTS = None  # BassKernelResults of the most recent run (for test.py)


def _build_bass():
    nc = bacc.Bacc("TRN2")
    lhs = nc.dram_tensor("lhs", [K, N], mybir.dt.bfloat16, kind="ExternalInput")
    rhs = nc.dram_tensor("rhs", [K, N], mybir.dt.bfloat16, kind="ExternalInput")
    n_s = sum(_is_soft(m) for m in range(NMB))
    out_s = nc.dram_tensor("out_s", [MBLK, n_s], mybir.dt.float32, kind="ExternalOutput")
    out_d = nc.dram_tensor("out_d", [MBLK, NMB - n_s], mybir.dt.float32, kind="ExternalOutput")
    mn = mybir.AluOpType.min

    with TileContext(nc) as tc:
        with (
            tc.tile_pool(name="data", bufs=1) as data_pool,
            tc.tile_pool(name="work", bufs=4) as work_pool,
            tc.tile_pool(name="ps", bufs=4, space="PSUM") as ps_pool,
        ):
            # operands replicated at partition offsets 0/32/64/96 so four
            # row-group-packed matmuls can run concurrently
            lhs_sb = data_pool.tile([128, N], mybir.dt.bfloat16)
            rhs_sb = data_pool.tile([128, N], mybir.dt.bfloat16)
            for g in range(4):
                nc.sync.dma_start(lhs_sb[32 * g : 32 * g + K, :], lhs.ap())
                nc.sync.dma_start(rhs_sb[32 * g : 32 * g + K, :], rhs.ap())

            arena_s = data_pool.tile([MBLK, n_s], mybir.dt.float32)
            arena_d = data_pool.tile([MBLK, NMB - n_s], mybir.dt.float32)
            i_s = i_d = 0

            for s in range(SB):
                for j in range(4):
                    m = 4 * s + j
                    lo = LOS[m]
                    nb = WS[m] // NBLK
                    ps = ps_pool.tile([MBLK, 2, NBLK], mybir.dt.float32, tag="ps")
                    for h in range(nb):
                        nc.tensor.matmul(
                            ps[:, h, :],
                            lhs_sb[32 * j : 32 * j + K, m * MBLK : (m + 1) * MBLK],
                            rhs_sb[32 * j : 32 * j + K, lo + h * NBLK : lo + (h + 1) * NBLK],
                            start=True,
                            stop=True,
                            tile_position=(32 * j, 0),
                        )
                    if _is_soft(m):  # S-block: ACT softmin (exp + sum-accum)
                        scratch = work_pool.tile(
                            [MBLK, 2, NBLK], mybir.dt.bfloat16, tag="sc"
                        )
                        nc.scalar.activation(
                            scratch[:, 0:nb],
                            ps[:, 0:nb],
                            mybir.ActivationFunctionType.Exp,
                            bias=0.0,
                            scale=-BETA,
                            accum_out=arena_s[:, i_s : i_s + 1],
                        )
                        i_s += 1
                    else:  # D-block: DVE exact min straight off PSUM
                        nc.vector.tensor_reduce(
                            arena_d[:, i_d : i_d + 1],
                            ps[:, 0:nb],
                            axis=mybir.AxisListType.XY,
                            op=mn,
                        )
                        i_d += 1

            nc.sync.dma_start(out_s.ap(), arena_s[:])
            nc.sync.dma_start(out_d.ap(), arena_d[:])
    return nc


def _split_bf16(v):
    """v (fp32) ~= hi + lo with both bf16; residual is O(2^-18 |v|)."""
    hi = v.astype(BF16)
    lo = (v - hi.astype(np.float32)).astype(BF16)
    return hi, lo


def _prep_core_inputs(Q, R):
    """Build the K=16 lhsT (queries) and rhs (refs) bf16 matrices so that
    lhsT.T @ rhs accumulated in fp32 equals |Q|^2 + |R|^2 - 2 Q.R."""
    Qh, Ql = _split_bf16(Q)  # [N, 3]
    Rh, Rl = _split_bf16(-2.0 * R)  # [N, 3]
    nQh, nQl = _split_bf16((Q * Q).sum(axis=1))  # [N]
    nRh, nRl = _split_bf16((R * R).sum(axis=1))  # [N]
    one = np.ones(N, dtype=BF16)

    L = np.empty([K, N], dtype=BF16)
    L[0:3] = Qh.T
    L[3:6] = Qh.T
    L[6:9] = Ql.T
    L[9:12] = Ql.T
    L[12] = nQh
    L[13] = nQl
    L[14] = one
    L[15] = one

    Rm = np.empty([K, N], dtype=BF16)
    Rm[0:3] = Rh.T
    Rm[3:6] = Rl.T
    Rm[6:9] = Rh.T
    Rm[9:12] = Rl.T
    Rm[12] = one
    Rm[13] = one
    Rm[14] = nRh
    Rm[15] = nRl
    return L, Rm


def _try_axon_reset():
    """The axon-tunneled device sporadically wedges (NRT_EXEC_UNIT_UNRECOVERABLE);
    axon_reset() recovers it."""
    try:
        import ctypes

        import jax

        jax.devices()
        lib = ctypes.CDLL("/opt/axon/libaxon_pjrt.so")
        lib.axon_reset.restype = ctypes.c_int64
        lib.axon_reset()
    except Exception:
        pass


def _task_pairs(gts_X, pred_X):
    for b in range(B):
        yield gts_X[b], pred_X[b]  # each gts point -> nearest pred
        yield pred_X[b], gts_X[b]  # each pred point -> nearest gts


def kernel(gts_X, pred_X, gts_normals=None, **_ignored):
    global LAST_RESULTS
    gts_X = np.asarray(gts_X, dtype=np.float32)
    pred_X = np.asarray(pred_X, dtype=np.float32)
    assert gts_X.shape == (B, N, 3) and pred_X.shape == (B, N, 3)

    in_maps = []
    sorted_pairs = []
    for Qr, Rr in _task_pairs(gts_X, pred_X):
        Qs = np.ascontiguousarray(Qr[np.argsort(Qr[:, 2], kind="stable")])
        Rs = np.ascontiguousarray(Rr[np.argsort(Rr[:, 2], kind="stable")])
        sorted_pairs.append((Qs, Rs))
        L, Rm = _prep_core_inputs(Qs, Rs)
        in_maps.append({"lhs": L, "rhs": Rm})

    nc = _build_bass()
    nc.finalize()
    res = None
    for attempt in range(3):
        try:
            res = run_bass_kernel_spmd(nc, in_maps, core_ids=list(range(8)))
            break
        except Exception:
            if attempt == 2:
                raise
            _try_axon_reset()
    LAST_RESULTS = res

    los = np.array(LOS)
    q_idx = np.arange(N)
    lo = los[q_idx // MBLK]  # per-query window start
    hi = lo + np.array(WS)[q_idx // MBLK]
    soft = np.array([_is_soft(m) for m in range(NMB)])[q_idx // MBLK]
    s_blocks = [m for m in range(NMB) if _is_soft(m)]
    d_blocks = [m for m in range(NMB) if not _is_soft(m)]

    total = 0.0
    for (Qs, Rs), r in zip(sorted_pairs, res.results):
        vals = np.empty((NMB, MBLK))  # [block, partition]; query rank = m*128+p
        vals[s_blocks] = r["out_s"].astype(np.float64).T
        vals[d_blocks] = r["out_d"].astype(np.float64).T
        vals = vals.reshape(-1)
        mins = np.where(
            soft,
            -np.log(np.maximum(vals, 1e-300)) / BETA,  # softmin recovery
            vals,
        )
        # certification: true NN outside the window only if the squared z-gap
        # to the window edge is below the windowed min (pad for softmin bias /
        # exp-table error); softmin underflow (tiny S) is also uncertified.
        zq = Qs[:, 2].astype(np.float64)
        zr = Rs[:, 2].astype(np.float64)
        gap_lo = np.where(lo > 0, zq - zr[np.maximum(lo - 1, 0)], np.inf)
        gap_hi = np.where(hi < N, zr[np.minimum(hi, N - 1)] - zq, np.inf)
        guard = np.minimum(gap_lo, gap_hi) ** 2
        bad = (mins > guard * (1.0 - 2.0**-7)) | (soft & (vals < S_MIN))
        bad = np.nonzero(bad)[0]
        if len(bad):
            Qb = Qs[bad].astype(np.float64)
            Rd = Rs.astype(np.float64)
            nq = (Qb * Qb).sum(1)
            nr = (Rd * Rd).sum(1)
            d = nq[:, None] + nr[None, :] - 2.0 * (Qb @ Rd.T)
            mins[bad] = d.min(axis=1)
        total += mins.sum()

    loss = total / (B * N)
    return np.asarray(loss, dtype=np.float32)


# revision 6
# speedup vs baseline: 1.9729x; 1.2319x over previous
"""Chamfer distance (pytorch3d defaults) on 8 Trainium2 NeuronCores.

Problem: gts_X, pred_X: [4, 8192, 3] fp32. loss = mean_b mean_n min_p d(x_bn, y_bp)
                                              + mean_b mean_p min_n d(x_bn, y_bp),
d = squared euclidean distance. gts_normals is unused (reference default path).

Sharding: 8 independent tasks = 4 batches x 2 directions, one per core.
Each core computes per-query windowed min over a 1024-wide, per-row-block
centered window of z-sorted refs; the host certifies each query with a z-gap
guard and recomputes the uncertified queries exactly in numpy.

Device algorithm per core (v2c):
- d[q, r] = |Q|^2 + |R|^2 - 2 Q.R via ONE K=16 bf16 matmul per (128q x 512r)
  tile using an exact hi/lo bf16 split (~fp32 precision in PSUM). Matmuls are
  packed 4x with tile_position row groups (keeps the PE at the 267ns/tile
  fused-weight-load pace; unpacked they cost 618+134ns).
- Per 128-query row block m: window = refs [lo_m, lo_m+1024) -> 2 matmuls
  into a [128, 2, 512] PSUM tile.
- PSUM drain (the wall: only DVE and ACT can read PSUM, ~1 elem/cycle/lane):
  - S-blocks (even m): ONE ACT op: out=exp(-BETA*d) with accum_out giving
    S_q = sum_r exp(-BETA * d_qr); the host recovers the windowed softmin
    -ln(S)/BETA (bias ~ -1e-5, validated under the 2e-2 tolerance; S==0 /
    tiny-S queries are recomputed exactly on host, as are guard escapes).
  - D-blocks (odd m): ONE DVE tensor_reduce XY straight off PSUM -> exact min.
  Each engine drains half the elements with zero cross-engine coupling.
"""

import sys

sys.path.insert(0, "/opt/trn_rl_repo")

import numpy as np
import ml_dtypes

import concourse.bacc as bacc
import concourse.mybir as mybir
from concourse.tile import TileContext
from concourse.bass_utils import run_bass_kernel_spmd

BF16 = ml_dtypes.bfloat16

B = 4
N = 8192
K = 13  # contraction rows after hi/lo split (ll cross term dropped)
MBLK = 128  # queries per row block (PSUM partitions)
NBLK = 512  # refs per matmul (one PSUM bank of fp32)
NMB = N // MBLK  # 64 row blocks
SB = NMB // 4  # 16 super-blocks of 4 row blocks
TAIL = 8  # blocks on each end that scan half-width windows
WS = [512 if (m < TAIL or m >= NMB - TAIL) else 1024 for m in range(NMB)]

# per-row-block window start (centered on the block's rank range)
LOS = [min(max(128 * m + 64 - WS[m] // 2, 0), N - WS[m]) for m in range(NMB)]

BETA = 2500.0  # softmin sharpness (squared-distance units)
S_MIN = float(np.exp(-75.0))  # below this the softmin is underflow-suspect


def _is_soft(m):
    return m % 2 == 0 and m not in (28, 36)


LAST_RESULTS = None  # BassKernelResults of the most recent run (for test.py)


def _build_bass():
    nc = bacc.Bacc("TRN2")
    lhs = nc.dram_tensor("lhs", [K, N], mybir.dt.bfloat16, kind="ExternalInput")
    rhs = nc.dram_tensor("rhs", [K, N], mybir.dt.bfloat16, kind="ExternalInput")
    n_s = sum(_is_soft(m) for m in range(NMB))
    out_s = nc.dram_tensor("out_s", [MBLK, n_s], mybir.dt.float32, kind="ExternalOutput")
    out_d = nc.dram_tensor("out_d", [MBLK, NMB - n_s], mybir.dt.float32, kind="ExternalOutput")
    mn = mybir.AluOpType.min

    with TileContext(nc) as tc:
        with (
            tc.tile_pool(name="data", bufs=1) as data_pool,
            tc.tile_pool(name="work", bufs=4) as work_pool,
            tc.tile_pool(name="ps", bufs=4, space="PSUM") as ps_pool,
        ):
            # operands replicated at partition offsets 0/32 (separate tiles,
            # column-chunked DMAs) so adjacent blocks' matmuls overlap in the
            # PE array and the first blocks start after ~1/16 of the input DMA
            l0 = data_pool.tile([K, N], mybir.dt.bfloat16)
            r0 = data_pool.tile([K, N], mybir.dt.bfloat16)
            l1 = data_pool.tile([32 + K, N], mybir.dt.bfloat16)
            r1 = data_pool.tile([32 + K, N], mybir.dt.bfloat16)
            CH = N // 4
            for c in range(4):
                cs = slice(c * CH, (c + 1) * CH)
                nc.sync.dma_start(l0[:, cs], lhs.ap()[:, cs])
                nc.sync.dma_start(r0[:, cs], rhs.ap()[:, cs])
                nc.sync.dma_start(l1[32 : 32 + K, cs], lhs.ap()[:, cs])
                nc.sync.dma_start(r1[32 : 32 + K, cs], rhs.ap()[:, cs])
            lrep = [l0, l1]
            rrep = [r0, r1]

            arena_s = data_pool.tile([MBLK, n_s], mybir.dt.float32)
            arena_d = data_pool.tile([MBLK, NMB - n_s], mybir.dt.float32)
            i_s = i_d = 0

            for s in range(SB):
                for j in range(4):
                    m = 4 * s + j
                    g = m % 2
                    po = 32 * g
                    lt, rt = lrep[g], rrep[g]
                    lo = LOS[m]
                    nb = WS[m] // NBLK
                    ps = ps_pool.tile([MBLK, 2, NBLK], mybir.dt.float32, tag="ps")
                    for h in range(nb):
                        nc.tensor.matmul(
                            ps[:, h, :],
                            lt[po : po + K, m * MBLK : (m + 1) * MBLK],
                            rt[po : po + K, lo + h * NBLK : lo + (h + 1) * NBLK],
                            start=True,
                            stop=True,
                            tile_position=(po, 0),
                        )
                    if _is_soft(m):  # S-block: ACT softmin (exp + sum-accum)
                        scratch = work_pool.tile(
                            [MBLK, 2, NBLK], mybir.dt.bfloat16, tag="sc"
                        )
                        nc.scalar.activation(
                            scratch[:, 0:nb],
                            ps[:, 0:nb],
                            mybir.ActivationFunctionType.Exp,
                            bias=0.0,
                            scale=-BETA,
                            accum_out=arena_s[:, i_s : i_s + 1],
                        )
                        i_s += 1
                    else:  # D-block: DVE exact min straight off PSUM
                        nc.vector.tensor_reduce(
                            arena_d[:, i_d : i_d + 1],
                            ps[:, 0:nb],
                            axis=mybir.AxisListType.XY,
                            op=mn,
                        )
                        i_d += 1

            nc.sync.dma_start(out_s.ap(), arena_s[:])
            nc.sync.dma_start(out_d.ap(), arena_d[:])
    return nc


def _split_bf16(v):
    """v (fp32) ~= hi + lo with both bf16; residual is O(2^-18 |v|)."""
    hi = v.astype(BF16)
    lo = (v - hi.astype(np.float32)).astype(BF16)
    return hi, lo


def _prep_core_inputs(Q, R):
    """Build the K=16 lhsT (queries) and rhs (refs) bf16 matrices so that
    lhsT.T @ rhs accumulated in fp32 equals |Q|^2 + |R|^2 - 2 Q.R."""
    Qh, Ql = _split_bf16(Q)  # [N, 3]
    Rh, Rl = _split_bf16(-2.0 * R)  # [N, 3]
    nQh, nQl = _split_bf16((Q * Q).sum(axis=1))  # [N]
    nRh, nRl = _split_bf16((R * R).sum(axis=1))  # [N]
    one = np.ones(N, dtype=BF16)

    L = np.empty([K, N], dtype=BF16)
    L[0:3] = Qh.T
    L[3:6] = Qh.T
    L[6:9] = Ql.T
    L[9] = nQh
    L[10] = nQl
    L[11] = one
    L[12] = one

    Rm = np.empty([K, N], dtype=BF16)
    Rm[0:3] = Rh.T
    Rm[3:6] = Rl.T
    Rm[6:9] = Rh.T
    Rm[9] = one
    Rm[10] = one
    Rm[11] = nRh
    Rm[12] = nRl
    return L, Rm


def _try_axon_reset():
    """The axon-tunneled device sporadically wedges (NRT_EXEC_UNIT_UNRECOVERABLE);
    axon_reset() recovers it."""
    try:
        import ctypes

        import jax

        jax.devices()
        lib = ctypes.CDLL("/opt/axon/libaxon_pjrt.so")
        lib.axon_reset.restype = ctypes.c_int64
        lib.axon_reset()
    except Exception:
        pass


def _task_pairs(gts_X, pred_X):
    for b in range(B):
        yield gts_X[b], pred_X[b]  # each gts point -> nearest pred
        yield pred_X[b], gts_X[b]  # each pred point -> nearest gts


def kernel(gts_X, pred_X, gts_normals=None, **_ignored):
    global LAST_RESULTS
    gts_X = np.asarray(gts_X, dtype=np.float32)
    pred_X = np.asarray(pred_X, dtype=np.float32)
    assert gts_X.shape == (B, N, 3) and pred_X.shape == (B, N, 3)

    in_maps = []
    sorted_pairs = []
    for Qr, Rr in _task_pairs(gts_X, pred_X):
        Qs = np.ascontiguousarray(Qr[np.argsort(Qr[:, 2], kind="stable")])
        Rs = np.ascontiguousarray(Rr[np.argsort(Rr[:, 2], kind="stable")])
        sorted_pairs.append((Qs, Rs))
        L, Rm = _prep_core_inputs(Qs, Rs)
        in_maps.append({"lhs": L, "rhs": Rm})

    nc = _build_bass()
    nc.finalize()
    res = None
    for attempt in range(3):
        try:
            res = run_bass_kernel_spmd(nc, in_maps, core_ids=list(range(8)))
            break
        except Exception:
            if attempt == 2:
                raise
            _try_axon_reset()
    LAST_RESULTS = res

    los = np.array(LOS)
    q_idx = np.arange(N)
    lo = los[q_idx // MBLK]  # per-query window start
    hi = lo + np.array(WS)[q_idx // MBLK]
    soft = np.array([_is_soft(m) for m in range(NMB)])[q_idx // MBLK]
    s_blocks = [m for m in range(NMB) if _is_soft(m)]
    d_blocks = [m for m in range(NMB) if not _is_soft(m)]

    total = 0.0
    for (Qs, Rs), r in zip(sorted_pairs, res.results):
        vals = np.empty((NMB, MBLK))  # [block, partition]; query rank = m*128+p
        vals[s_blocks] = r["out_s"].astype(np.float64).T
        vals[d_blocks] = r["out_d"].astype(np.float64).T
        vals = vals.reshape(-1)
        mins = np.where(
            soft,
            -np.log(np.maximum(vals, 1e-300)) / BETA,  # softmin recovery
            vals,
        )
        # certification: true NN outside the window only if the squared z-gap
        # to the window edge is below the windowed min (pad for softmin bias /
        # exp-table error); softmin underflow (tiny S) is also uncertified.
        zq = Qs[:, 2].astype(np.float64)
        zr = Rs[:, 2].astype(np.float64)
        gap_lo = np.where(lo > 0, zq - zr[np.maximum(lo - 1, 0)], np.inf)
        gap_hi = np.where(hi < N, zr[np.minimum(hi, N - 1)] - zq, np.inf)
        guard = np.minimum(gap_lo, gap_hi) ** 2
        bad = (mins > guard * (1.0 - 2.0**-7)) | (soft & (vals < S_MIN))
        bad = np.nonzero(bad)[0]
        if len(bad):
            Qb = Qs[bad].astype(np.float64)
            Rd = Rs.astype(np.float64)
            nq = (Qb * Qb).sum(1)
            nr = (Rd * Rd).sum(1)
            d = nq[:, None] + nr[None, :] - 2.0 * (Qb @ Rd.T)
            mins[bad] = d.min(axis=1)
        total += mins.sum()

    loss = total / (B * N)
    return np.asarray(loss, dtype=np.float32)


# revision 8
# speedup vs baseline: 1.9920x; 1.0097x over previous
"""Chamfer distance (pytorch3d defaults) on 8 Trainium2 NeuronCores.

Problem: gts_X, pred_X: [4, 8192, 3] fp32. loss = mean_b mean_n min_p d(x_bn, y_bp)
                                              + mean_b mean_p min_n d(x_bn, y_bp),
d = squared euclidean distance. gts_normals is unused (reference default path).

Sharding: 8 independent tasks = 4 batches x 2 directions, one per core.
Each core computes per-query windowed min over a 1024-wide, per-row-block
centered window of z-sorted refs; the host certifies each query with a z-gap
guard and recomputes the uncertified queries exactly in numpy.

Device algorithm per core (v2c):
- d[q, r] = |Q|^2 + |R|^2 - 2 Q.R via ONE K=16 bf16 matmul per (128q x 512r)
  tile using an exact hi/lo bf16 split (~fp32 precision in PSUM). Matmuls are
  packed 4x with tile_position row groups (keeps the PE at the 267ns/tile
  fused-weight-load pace; unpacked they cost 618+134ns).
- Per 128-query row block m: window = refs [lo_m, lo_m+1024) -> 2 matmuls
  into a [128, 2, 512] PSUM tile.
- PSUM drain (the wall: only DVE and ACT can read PSUM, ~1 elem/cycle/lane):
  - S-blocks (even m): ONE ACT op: out=exp(-BETA*d) with accum_out giving
    S_q = sum_r exp(-BETA * d_qr); the host recovers the windowed softmin
    -ln(S)/BETA (bias ~ -1e-5, validated under the 2e-2 tolerance; S==0 /
    tiny-S queries are recomputed exactly on host, as are guard escapes).
  - D-blocks (odd m): ONE DVE tensor_reduce XY straight off PSUM -> exact min.
  Each engine drains half the elements with zero cross-engine coupling.
"""

import sys

sys.path.insert(0, "/opt/trn_rl_repo")

import numpy as np
import ml_dtypes

import concourse.bacc as bacc
import concourse.mybir as mybir
from concourse.tile import TileContext
from concourse.bass_utils import run_bass_kernel_spmd

BF16 = ml_dtypes.bfloat16

B = 4
N = 8192
K = 13  # contraction rows after hi/lo split (ll cross term dropped)
MBLK = 128  # queries per row block (PSUM partitions)
NBLK = 512  # refs per matmul (one PSUM bank of fp32)
NMB = N // MBLK  # 64 row blocks
SB = NMB // 4  # 16 super-blocks of 4 row blocks
TAIL = 8  # blocks on each end that scan half-width windows
WS = [512 if (m < TAIL or m >= NMB - TAIL) else 1024 for m in range(NMB)]

# per-row-block window start (centered on the block's rank range)
LOS = [min(max(128 * m + 64 - WS[m] // 2, 0), N - WS[m]) for m in range(NMB)]

BETA = 2500.0  # softmin sharpness (squared-distance units)
S_MIN = float(np.exp(-75.0))  # below this the softmin is underflow-suspect


def _is_soft(m):
    return m % 2 == 0 and m not in (28, 36)


LAST_RESULTS = None  # BassKernelResults of the most recent run (for test.py)


def _build_bass():
    nc = bacc.Bacc("TRN2")
    lhs = nc.dram_tensor("lhs", [K, N], mybir.dt.bfloat16, kind="ExternalInput")
    rhs = nc.dram_tensor("rhs", [K, N], mybir.dt.bfloat16, kind="ExternalInput")
    n_s = sum(_is_soft(m) for m in range(NMB))
    out_s = nc.dram_tensor("out_s", [MBLK, n_s], mybir.dt.float32, kind="ExternalOutput")
    out_d = nc.dram_tensor("out_d", [MBLK, NMB - n_s], mybir.dt.float32, kind="ExternalOutput")
    mn = mybir.AluOpType.min

    with TileContext(nc) as tc:
        with (
            tc.tile_pool(name="data", bufs=1) as data_pool,
            tc.tile_pool(name="work", bufs=4) as work_pool,
            tc.tile_pool(name="ps", bufs=4, space="PSUM") as ps_pool,
        ):
            # operands replicated at partition offsets 0/32 (separate tiles,
            # column-chunked DMAs) so adjacent blocks' matmuls overlap in the
            # PE array and the first blocks start after ~1/16 of the input DMA
            l0 = data_pool.tile([K, N], mybir.dt.bfloat16)
            r0 = data_pool.tile([K, N], mybir.dt.bfloat16)
            l1 = data_pool.tile([32 + K, N], mybir.dt.bfloat16)
            r1 = data_pool.tile([32 + K, N], mybir.dt.bfloat16)
            CH = N // 8
            for c in range(8):
                cs = slice(c * CH, (c + 1) * CH)
                nc.sync.dma_start(l0[:, cs], lhs.ap()[:, cs])
                nc.sync.dma_start(r0[:, cs], rhs.ap()[:, cs])
                nc.sync.dma_start(l1[32 : 32 + K, cs], lhs.ap()[:, cs])
                nc.sync.dma_start(r1[32 : 32 + K, cs], rhs.ap()[:, cs])
            lrep = [l0, l1]
            rrep = [r0, r1]

            arena_s = data_pool.tile([MBLK, n_s], mybir.dt.float32)
            arena_d = data_pool.tile([MBLK, NMB - n_s], mybir.dt.float32)
            i_s = i_d = 0

            for s in range(SB):
                for j in range(4):
                    m = 4 * s + j
                    g = m % 2
                    po = 32 * g
                    lt, rt = lrep[g], rrep[g]
                    lo = LOS[m]
                    nb = WS[m] // NBLK
                    ps = ps_pool.tile([MBLK, 2, NBLK], mybir.dt.float32, tag="ps")
                    for h in range(nb):
                        nc.tensor.matmul(
                            ps[:, h, :],
                            lt[po : po + K, m * MBLK : (m + 1) * MBLK],
                            rt[po : po + K, lo + h * NBLK : lo + (h + 1) * NBLK],
                            start=True,
                            stop=True,
                            tile_position=(po, 0),
                        )
                    if _is_soft(m):  # S-block: ACT softmin (exp + sum-accum)
                        scratch = work_pool.tile(
                            [MBLK, 2, NBLK], mybir.dt.bfloat16, tag="sc"
                        )
                        nc.scalar.activation(
                            scratch[:, 0:nb],
                            ps[:, 0:nb],
                            mybir.ActivationFunctionType.Exp,
                            bias=0.0,
                            scale=-BETA,
                            accum_out=arena_s[:, i_s : i_s + 1],
                        )
                        i_s += 1
                    else:  # D-block: DVE exact min straight off PSUM
                        nc.vector.tensor_reduce(
                            arena_d[:, i_d : i_d + 1],
                            ps[:, 0:nb],
                            axis=mybir.AxisListType.XY,
                            op=mn,
                        )
                        i_d += 1

            nc.sync.dma_start(out_s.ap(), arena_s[:])
            nc.sync.dma_start(out_d.ap(), arena_d[:])
    return nc


def _split_bf16(v):
    """v (fp32) ~= hi + lo with both bf16; residual is O(2^-18 |v|)."""
    hi = v.astype(BF16)
    lo = (v - hi.astype(np.float32)).astype(BF16)
    return hi, lo


def _prep_core_inputs(Q, R):
    """Build the K=16 lhsT (queries) and rhs (refs) bf16 matrices so that
    lhsT.T @ rhs accumulated in fp32 equals |Q|^2 + |R|^2 - 2 Q.R."""
    Qh, Ql = _split_bf16(Q)  # [N, 3]
    Rh, Rl = _split_bf16(-2.0 * R)  # [N, 3]
    nQh, nQl = _split_bf16((Q * Q).sum(axis=1))  # [N]
    nRh, nRl = _split_bf16((R * R).sum(axis=1))  # [N]
    one = np.ones(N, dtype=BF16)

    L = np.empty([K, N], dtype=BF16)
    L[0:3] = Qh.T
    L[3:6] = Qh.T
    L[6:9] = Ql.T
    L[9] = nQh
    L[10] = nQl
    L[11] = one
    L[12] = one

    Rm = np.empty([K, N], dtype=BF16)
    Rm[0:3] = Rh.T
    Rm[3:6] = Rl.T
    Rm[6:9] = Rh.T
    Rm[9] = one
    Rm[10] = one
    Rm[11] = nRh
    Rm[12] = nRl
    return L, Rm


def _try_axon_reset():
    """The axon-tunneled device sporadically wedges (NRT_EXEC_UNIT_UNRECOVERABLE);
    axon_reset() recovers it."""
    try:
        import ctypes

        import jax

        jax.devices()
        lib = ctypes.CDLL("/opt/axon/libaxon_pjrt.so")
        lib.axon_reset.restype = ctypes.c_int64
        lib.axon_reset()
    except Exception:
        pass


def _task_pairs(gts_X, pred_X):
    for b in range(B):
        yield gts_X[b], pred_X[b]  # each gts point -> nearest pred
        yield pred_X[b], gts_X[b]  # each pred point -> nearest gts


def kernel(gts_X, pred_X, gts_normals=None, **_ignored):
    global LAST_RESULTS
    gts_X = np.asarray(gts_X, dtype=np.float32)
    pred_X = np.asarray(pred_X, dtype=np.float32)
    assert gts_X.shape == (B, N, 3) and pred_X.shape == (B, N, 3)

    in_maps = []
    sorted_pairs = []
    for Qr, Rr in _task_pairs(gts_X, pred_X):
        Qs = np.ascontiguousarray(Qr[np.argsort(Qr[:, 2], kind="stable")])
        Rs = np.ascontiguousarray(Rr[np.argsort(Rr[:, 2], kind="stable")])
        sorted_pairs.append((Qs, Rs))
        L, Rm = _prep_core_inputs(Qs, Rs)
        in_maps.append({"lhs": L, "rhs": Rm})

    nc = _build_bass()
    nc.finalize()
    res = None
    for attempt in range(3):
        try:
            res = run_bass_kernel_spmd(nc, in_maps, core_ids=list(range(8)))
            break
        except Exception:
            if attempt == 2:
                raise
            _try_axon_reset()
    LAST_RESULTS = res

    los = np.array(LOS)
    q_idx = np.arange(N)
    lo = los[q_idx // MBLK]  # per-query window start
    hi = lo + np.array(WS)[q_idx // MBLK]
    soft = np.array([_is_soft(m) for m in range(NMB)])[q_idx // MBLK]
    s_blocks = [m for m in range(NMB) if _is_soft(m)]
    d_blocks = [m for m in range(NMB) if not _is_soft(m)]

    total = 0.0
    for (Qs, Rs), r in zip(sorted_pairs, res.results):
        vals = np.empty((NMB, MBLK))  # [block, partition]; query rank = m*128+p
        vals[s_blocks] = r["out_s"].astype(np.float64).T
        vals[d_blocks] = r["out_d"].astype(np.float64).T
        vals = vals.reshape(-1)
        mins = np.where(
            soft,
            -np.log(np.maximum(vals, 1e-300)) / BETA,  # softmin recovery
            vals,
        )
        # certification: true NN outside the window only if the squared z-gap
        # to the window edge is below the windowed min (pad for softmin bias /
        # exp-table error); softmin underflow (tiny S) is also uncertified.
        zq = Qs[:, 2].astype(np.float64)
        zr = Rs[:, 2].astype(np.float64)
        gap_lo = np.where(lo > 0, zq - zr[np.maximum(lo - 1, 0)], np.inf)
        gap_hi = np.where(hi < N, zr[np.minimum(hi, N - 1)] - zq, np.inf)
        guard = np.minimum(gap_lo, gap_hi) ** 2
        bad = (mins > guard * (1.0 - 2.0**-7)) | (soft & (vals < S_MIN))
        bad = np.nonzero(bad)[0]
        if len(bad):
            Qb = Qs[bad].astype(np.float64)
            Rd = Rs.astype(np.float64)
            nq = (Qb * Qb).sum(1)
            nr = (Rd * Rd).sum(1)
            d = nq[:, None] + nr[None, :] - 2.0 * (Qb @ Rd.T)
            mins[bad] = d.min(axis=1)
        total += mins.sum()

    loss = total / (B * N)
    return np.asarray(loss, dtype=np.float32)


# revision 9
# speedup vs baseline: 2.2216x; 1.1153x over previous
"""Chamfer distance (pytorch3d defaults) on 8 Trainium2 NeuronCores.

Problem: gts_X, pred_X: [4, 8192, 3] fp32. loss = mean_b mean_n min_p d(x_bn, y_bp)
                                              + mean_b mean_p min_n d(x_bn, y_bp),
d = squared euclidean distance. gts_normals is unused (reference default path).

Sharding: 8 independent tasks = 4 batches x 2 directions, one per core.
Each core computes per-query windowed min over a 1024-wide, per-row-block
centered window of z-sorted refs; the host certifies each query with a z-gap
guard and recomputes the uncertified queries exactly in numpy.

Device algorithm per core (v2c):
- d[q, r] = |Q|^2 + |R|^2 - 2 Q.R via ONE K=16 bf16 matmul per (128q x 512r)
  tile using an exact hi/lo bf16 split (~fp32 precision in PSUM). Matmuls are
  packed 4x with tile_position row groups (keeps the PE at the 267ns/tile
  fused-weight-load pace; unpacked they cost 618+134ns).
- Per 128-query row block m: window = refs [lo_m, lo_m+1024) -> 2 matmuls
  into a [128, 2, 512] PSUM tile.
- PSUM drain (the wall: only DVE and ACT can read PSUM, ~1 elem/cycle/lane):
  - S-blocks (even m): ONE ACT op: out=exp(-BETA*d) with accum_out giving
    S_q = sum_r exp(-BETA * d_qr); the host recovers the windowed softmin
    -ln(S)/BETA (bias ~ -1e-5, validated under the 2e-2 tolerance; S==0 /
    tiny-S queries are recomputed exactly on host, as are guard escapes).
  - D-blocks (odd m): ONE DVE tensor_reduce XY straight off PSUM -> exact min.
  Each engine drains half the elements with zero cross-engine coupling.
"""

import sys

sys.path.insert(0, "/opt/trn_rl_repo")

import numpy as np
import ml_dtypes

import concourse.bacc as bacc
import concourse.mybir as mybir
from concourse.tile import TileContext
from concourse.bass_utils import run_bass_kernel_spmd

BF16 = ml_dtypes.bfloat16

B = 4
N = 8192
K = 13  # contraction rows after hi/lo split (ll cross term dropped)
MBLK = 128  # queries per row block (PSUM partitions)
NBLK = 512  # refs per matmul (one PSUM bank of fp32)
NMB = N // MBLK  # 64 row blocks
SB = NMB // 4  # 16 super-blocks of 4 row blocks
TAIL = 8  # blocks on each end that scan half-width windows
WS = [512 if (m < TAIL or m >= NMB - TAIL) else 768 for m in range(NMB)]

# per-row-block window start (centered on the block's rank range)
LOS = [min(max(128 * m + 64 - WS[m] // 2, 0), N - WS[m]) for m in range(NMB)]

BETA = 2500.0  # softmin sharpness (squared-distance units)
S_MIN = float(np.exp(-75.0))  # below this the softmin is underflow-suspect


def _is_soft(m):
    return m % 2 == 0 and m not in (28, 36)


LAST_RESULTS = None  # BassKernelResults of the most recent run (for test.py)


def _build_bass():
    nc = bacc.Bacc("TRN2")
    lhs = nc.dram_tensor("lhs", [K, N], mybir.dt.bfloat16, kind="ExternalInput")
    rhs = nc.dram_tensor("rhs", [K, N], mybir.dt.bfloat16, kind="ExternalInput")
    n_s = sum(_is_soft(m) for m in range(NMB))
    out_s = nc.dram_tensor("out_s", [MBLK, n_s], mybir.dt.float32, kind="ExternalOutput")
    out_d = nc.dram_tensor("out_d", [MBLK, NMB - n_s], mybir.dt.float32, kind="ExternalOutput")
    mn = mybir.AluOpType.min

    with TileContext(nc) as tc:
        with (
            tc.tile_pool(name="data", bufs=1) as data_pool,
            tc.tile_pool(name="work", bufs=4) as work_pool,
            tc.tile_pool(name="ps", bufs=4, space="PSUM") as ps_pool,
        ):
            # operands replicated at partition offsets 0/32 (separate tiles,
            # column-chunked DMAs) so adjacent blocks' matmuls overlap in the
            # PE array and the first blocks start after ~1/16 of the input DMA
            l0 = data_pool.tile([K, N], mybir.dt.bfloat16)
            r0 = data_pool.tile([K, N], mybir.dt.bfloat16)
            l1 = data_pool.tile([32 + K, N], mybir.dt.bfloat16)
            r1 = data_pool.tile([32 + K, N], mybir.dt.bfloat16)
            CH = N // 8
            for c in range(8):
                cs = slice(c * CH, (c + 1) * CH)
                nc.sync.dma_start(l0[:, cs], lhs.ap()[:, cs])
                nc.sync.dma_start(r0[:, cs], rhs.ap()[:, cs])
                nc.sync.dma_start(l1[32 : 32 + K, cs], lhs.ap()[:, cs])
                nc.sync.dma_start(r1[32 : 32 + K, cs], rhs.ap()[:, cs])
            lrep = [l0, l1]
            rrep = [r0, r1]

            arena_s = data_pool.tile([MBLK, n_s], mybir.dt.float32)
            arena_d = data_pool.tile([MBLK, NMB - n_s], mybir.dt.float32)
            i_s = i_d = 0

            for s in range(SB):
                for j in range(4):
                    m = 4 * s + j
                    g = m % 2
                    po = 32 * g
                    lt, rt = lrep[g], rrep[g]
                    lo = LOS[m]
                    w = WS[m]
                    ps = ps_pool.tile([MBLK, 2, NBLK], mybir.dt.float32, tag="ps")
                    psf = ps[:].rearrange("p a b -> p (a b)")
                    off = 0
                    while off < w:
                        cw = min(NBLK - off % NBLK, w - off)
                        nc.tensor.matmul(
                            psf[:, off : off + cw],
                            lt[po : po + K, m * MBLK : (m + 1) * MBLK],
                            rt[po : po + K, lo + off : lo + off + cw],
                            start=True,
                            stop=True,
                            tile_position=(po, 0),
                        )
                        off += cw
                    if _is_soft(m):  # S-block: ACT softmin (exp + sum-accum)
                        scratch = work_pool.tile(
                            [MBLK, 2, NBLK], mybir.dt.bfloat16, tag="sc"
                        )
                        nc.scalar.activation(
                            scratch[:].rearrange("p a b -> p (a b)")[:, 0:w],
                            psf[:, 0:w],
                            mybir.ActivationFunctionType.Exp,
                            bias=0.0,
                            scale=-BETA,
                            accum_out=arena_s[:, i_s : i_s + 1],
                        )
                        i_s += 1
                    else:  # D-block: DVE exact min straight off PSUM
                        nc.vector.tensor_reduce(
                            arena_d[:, i_d : i_d + 1],
                            psf[:, 0:w],
                            axis=mybir.AxisListType.X,
                            op=mn,
                        )
                        i_d += 1

            nc.sync.dma_start(out_s.ap(), arena_s[:])
            nc.sync.dma_start(out_d.ap(), arena_d[:])
    return nc


def _split_bf16(v):
    """v (fp32) ~= hi + lo with both bf16; residual is O(2^-18 |v|)."""
    hi = v.astype(BF16)
    lo = (v - hi.astype(np.float32)).astype(BF16)
    return hi, lo


def _prep_core_inputs(Q, R):
    """Build the K=16 lhsT (queries) and rhs (refs) bf16 matrices so that
    lhsT.T @ rhs accumulated in fp32 equals |Q|^2 + |R|^2 - 2 Q.R."""
    Qh, Ql = _split_bf16(Q)  # [N, 3]
    Rh, Rl = _split_bf16(-2.0 * R)  # [N, 3]
    nQh, nQl = _split_bf16((Q * Q).sum(axis=1))  # [N]
    nRh, nRl = _split_bf16((R * R).sum(axis=1))  # [N]
    one = np.ones(N, dtype=BF16)

    L = np.empty([K, N], dtype=BF16)
    L[0:3] = Qh.T
    L[3:6] = Qh.T
    L[6:9] = Ql.T
    L[9] = nQh
    L[10] = nQl
    L[11] = one
    L[12] = one

    Rm = np.empty([K, N], dtype=BF16)
    Rm[0:3] = Rh.T
    Rm[3:6] = Rl.T
    Rm[6:9] = Rh.T
    Rm[9] = one
    Rm[10] = one
    Rm[11] = nRh
    Rm[12] = nRl
    return L, Rm


def _try_axon_reset():
    """The axon-tunneled device sporadically wedges (NRT_EXEC_UNIT_UNRECOVERABLE);
    axon_reset() recovers it."""
    try:
        import ctypes

        import jax

        jax.devices()
        lib = ctypes.CDLL("/opt/axon/libaxon_pjrt.so")
        lib.axon_reset.restype = ctypes.c_int64
        lib.axon_reset()
    except Exception:
        pass


def _task_pairs(gts_X, pred_X):
    for b in range(B):
        yield gts_X[b], pred_X[b]  # each gts point -> nearest pred
        yield pred_X[b], gts_X[b]  # each pred point -> nearest gts


def kernel(gts_X, pred_X, gts_normals=None, **_ignored):
    global LAST_RESULTS
    gts_X = np.asarray(gts_X, dtype=np.float32)
    pred_X = np.asarray(pred_X, dtype=np.float32)
    assert gts_X.shape == (B, N, 3) and pred_X.shape == (B, N, 3)

    in_maps = []
    sorted_pairs = []
    for Qr, Rr in _task_pairs(gts_X, pred_X):
        Qs = np.ascontiguousarray(Qr[np.argsort(Qr[:, 2], kind="stable")])
        Rs = np.ascontiguousarray(Rr[np.argsort(Rr[:, 2], kind="stable")])
        sorted_pairs.append((Qs, Rs))
        L, Rm = _prep_core_inputs(Qs, Rs)
        in_maps.append({"lhs": L, "rhs": Rm})

    nc = _build_bass()
    nc.finalize()
    res = None
    for attempt in range(3):
        try:
            res = run_bass_kernel_spmd(nc, in_maps, core_ids=list(range(8)))
            break
        except Exception:
            if attempt == 2:
                raise
            _try_axon_reset()
    LAST_RESULTS = res

    los = np.array(LOS)
    q_idx = np.arange(N)
    lo = los[q_idx // MBLK]  # per-query window start
    hi = lo + np.array(WS)[q_idx // MBLK]
    soft = np.array([_is_soft(m) for m in range(NMB)])[q_idx // MBLK]
    s_blocks = [m for m in range(NMB) if _is_soft(m)]
    d_blocks = [m for m in range(NMB) if not _is_soft(m)]

    total = 0.0
    for (Qs, Rs), r in zip(sorted_pairs, res.results):
        vals = np.empty((NMB, MBLK))  # [block, partition]; query rank = m*128+p
        vals[s_blocks] = r["out_s"].astype(np.float64).T
        vals[d_blocks] = r["out_d"].astype(np.float64).T
        vals = vals.reshape(-1)
        mins = np.where(
            soft,
            -np.log(np.maximum(vals, 1e-300)) / BETA,  # softmin recovery
            vals,
        )
        # certification: true NN outside the window only if the squared z-gap
        # to the window edge is below the windowed min (pad for softmin bias /
        # exp-table error); softmin underflow (tiny S) is also uncertified.
        zq = Qs[:, 2].astype(np.float64)
        zr = Rs[:, 2].astype(np.float64)
        gap_lo = np.where(lo > 0, zq - zr[np.maximum(lo - 1, 0)], np.inf)
        gap_hi = np.where(hi < N, zr[np.minimum(hi, N - 1)] - zq, np.inf)
        guard = np.minimum(gap_lo, gap_hi) ** 2
        bad = (mins > guard * (1.0 - 2.0**-7)) | (soft & (vals < S_MIN))
        bad = np.nonzero(bad)[0]
        if len(bad):
            Qb = Qs[bad].astype(np.float64)
            Rd = Rs.astype(np.float64)
            nq = (Qb * Qb).sum(1)
            nr = (Rd * Rd).sum(1)
            d = nq[:, None] + nr[None, :] - 2.0 * (Qb @ Rd.T)
            mins[bad] = d.min(axis=1)
        total += mins.sum()

    loss = total / (B * N)
    return np.asarray(loss, dtype=np.float32)
